# revision 5
# baseline (speedup 1.0000x reference)
"""Masked L1 loss (per-(b,c) normalized) on 8 Trainium2 NeuronCores.

Layout: batch-dim data parallel. Core i takes batches [2i, 2i+2) of the
[16, 64, 128, 128] inputs -> a [128, 16384] shard (partition = (b, c) pair,
free = h*w). DMA-bound (~24 MiB/core HBM reads), so the kernel is built
around keeping the DMA engines busy end-to-end and keeping the post-last-DMA
tail short:

  - 9 tiles of sizes 7x2048 + 1536 + 512; the small final tiles shrink the
    compute chain that remains after the last byte lands.
  - mask DMAs lag pre/gt by one tile so sub/abs of the last tile overlap the
    final mask transfers; the only op gated by the very last DMA is one
    512-wide STT.
  - per tile: DVE sub -> ACT abs -> DVE scalar_tensor_tensor (mult by mask,
    accum -> l1 partial). Counts (sum of 0/1 mask) go to DVE tensor_reduce
    for early tiles and ACT activation(Copy, accum_out) for late tiles,
    keeping both engines ~50% loaded and the DVE tail free.
  - DVE stream is software-pipelined (sub_{i+1} issued before stt_i) so the
    ACT abs latency never stalls DVE.
Host: loss = sum(l1 / max(ct, 1)) / B.
"""

import sys

if "/opt/trn_rl_repo" not in sys.path:
    sys.path.insert(0, "/opt/trn_rl_repo")

import numpy as np

B, C, H, W = 16, 64, 128, 128
N_CORES = 8
BPC = B // N_CORES          # batches per core = 2
P = BPC * C                 # partitions per core = 128 (one (b,c) pair each)
HW = H * W                  # 16384 free elements per partition

SIZES = [2048] * 7 + [864, 672, 512]   # sum = 16384; geometric tail
NT = len(SIZES)
OFFS = [sum(SIZES[:i]) for i in range(NT)]
N_DVE_COUNT = 4             # tiles [0, N_DVE_COUNT) count on DVE, rest on ACT

_CACHE = {}


def _build():
    key = "nc"
    if key in _CACHE:
        return _CACHE[key]

    import concourse.bacc as bacc
    import concourse.mybir as mybir
    from concourse.tile import TileContext

    f32 = mybir.dt.float32
    Alu = mybir.AluOpType
    Act = mybir.ActivationFunctionType

    nc = bacc.Bacc(
        "TRN2",
        target_bir_lowering=False,
        debug=False,
        enable_asserts=False,
        num_devices=N_CORES,
    )

    pre = nc.dram_tensor("pre", [P, HW], f32, kind="ExternalInput").ap()
    gt = nc.dram_tensor("gt", [P, HW], f32, kind="ExternalInput").ap()
    mask = nc.dram_tensor("mask", [P, HW], f32, kind="ExternalInput").ap()
    out = nc.dram_tensor("out", [P, 2 * NT], f32, kind="ExternalOutput").ap()

    with TileContext(nc) as tc:
        with (
            tc.tile_pool(name="pg", bufs=3) as pg,
            tc.tile_pool(name="mp", bufs=3) as mp,
            tc.tile_pool(name="work", bufs=3) as work,
            tc.tile_pool(name="acc", bufs=1) as accp,
        ):
            acc = accp.tile([P, 2 * NT], f32, tag="acc")
            l1p = acc[:, 0:NT]
            ctp = acc[:, NT : 2 * NT]
            trash = accp.tile([P, 2048], f32, tag="trash")

            tm = [None] * NT
            ad = [None] * NT

            for i in range(NT + 1):
                # input DMAs: pre/gt for tile i, mask lagging one tile
                if i < NT:
                    s, o = SIZES[i], OFFS[i]
                    tp = pg.tile([P, s], f32, tag="pre")
                    tg = pg.tile([P, s], f32, tag="gt")
                    nc.sync.dma_start(out=tp, in_=pre[:, o : o + s])
                    nc.sync.dma_start(out=tg, in_=gt[:, o : o + s])
                if i >= 1:
                    j = i - 1
                    sj, oj = SIZES[j], OFFS[j]
                    tm[j] = mp.tile([P, sj], f32, tag="mask", name=f"tm{j}")
                    nc.sync.dma_start(out=tm[j], in_=mask[:, oj : oj + sj])

                # compute: sub/abs for tile i; stt/count for tile i-1
                if i < NT:
                    sd = work.tile([P, s], f32, tag="sd")
                    ad[i] = work.tile([P, s], f32, tag="ad", name=f"ad{i}")
                    nc.vector.tensor_tensor(out=sd, in0=tp, in1=tg, op=Alu.subtract)
                    nc.scalar.activation(out=ad[i], in_=sd, func=Act.Abs)
                if i >= 1:
                    j = i - 1
                    sj = SIZES[j]
                    junk = work.tile([P, sj], f32, tag="junk")
                    # one DVE pass: junk = ad * mask, l1 partial = sum(junk)
                    nc.vector.scalar_tensor_tensor(
                        out=junk,
                        in0=ad[j],
                        scalar=0.0,
                        in1=tm[j],
                        op0=Alu.bypass,
                        op1=Alu.mult,
                        accum_out=l1p[:, j : j + 1],
                    )
                    # mask is 0/1 so sum(mask) == nonzero count
                    if j < N_DVE_COUNT:
                        nc.vector.tensor_reduce(
                            out=ctp[:, j : j + 1],
                            in_=tm[j],
                            axis=mybir.AxisListType.X,
                            op=Alu.add,
                        )
                    else:
                        nc.scalar.activation(
                            out=trash[:, :sj],
                            in_=tm[j],
                            func=Act.Copy,
                            accum_out=ctp[:, j : j + 1],
                        )

            nc.sync.dma_start(out=out, in_=acc)

    nc.compile()
    _CACHE[key] = nc
    return nc


def _shard(pre, gt, mask):
    in_maps = []
    for i in range(N_CORES):
        sl = slice(i * BPC, (i + 1) * BPC)
        in_maps.append(
            {
                "pre": np.ascontiguousarray(pre[sl], dtype=np.float32).reshape(P, HW),
                "gt": np.ascontiguousarray(gt[sl], dtype=np.float32).reshape(P, HW),
                "mask": np.ascontiguousarray(mask[sl], dtype=np.float32).reshape(P, HW),
            }
        )
    return in_maps


def _combine(results, batch_size):
    total = np.float32(0.0)
    for r in results:
        o = np.asarray(r["out"], dtype=np.float32)
        nt = o.shape[1] // 2
        l1 = o[:, :nt].sum(axis=1, dtype=np.float32)
        ct = o[:, nt:].sum(axis=1, dtype=np.float32)
        total += (l1 / np.maximum(ct, np.float32(1.0))).sum(dtype=np.float32)
    return np.asarray(total / np.float32(int(batch_size)), dtype=np.float32)


def run(pre, gt, mask, batch_size, trace=False, **bass_kwargs):
    from concourse.bass_utils import run_bass_kernel_spmd

    nc = _build()
    in_maps = _shard(np.asarray(pre), np.asarray(gt), np.asarray(mask))
    res = run_bass_kernel_spmd(
        nc, in_maps, list(range(N_CORES)), trace=trace, **bass_kwargs
    )
    loss = _combine(res.results, batch_size)
    return loss, res


def kernel(pre, gt, mask, batch_size):
    loss, _ = run(pre, gt, mask, batch_size)
    return loss


# revision 19
# speedup vs baseline: 1.7216x; 1.7216x over previous
"""Masked L1 loss (per-(b,c) normalized) on 8 Trainium2 NeuronCores.

Layout: batch-dim data parallel. Core i takes batches [2i, 2i+2) of the
[16, 64, 128, 128] inputs -> a [128, 16384] shard (partition = (b, c) pair,
free = h*w). The kernel is DMA-bound, so everything is built around the DMA
stream:

  - pre and gt are repacked on the host into one tile-major tensor
    (per tile: [pre_i | gt_i] blocks), so each tile needs ONE pre/gt DMA
    plus one mask DMA -> 19 input DMAs. All input DMAs use the gpsimd SWDGE
    path (the only one that can downcast in flight); fewer DMAs matter
    because SWDGE descriptor generation is serial (~1 us per DMA).
  - all inputs are DMA'd with an inline f32 -> bf16 cast: HBM reads are
    unchanged but the SBUF-side stream halves and all on-chip operands are
    16-bit, which doubles DVE tensor_tensor throughput (2x mode). Loss
    tolerance is 2e-2; bf16 + fp32 accumulation gives ~1e-4.
  - per tile, DVE runs just two 2x-mode tensor_tensor ops (sd = pre - gt,
    y = sd * mask); the l1 partial sum(|y|) = sum(|d|*mask) comes from ACT
    activation(Abs, accum_out) consuming y. ACT is a pure sink - nothing
    downstream waits on it mid-stream - so the cross-engine hop never
    stalls the pipeline. The last two (small) tiles instead use the all-DVE
    tensor_reduce(add, apply_absolute_value) so the post-last-byte chain
    stays on one engine.
  - the bf16 mask stays resident in SBUF (32 KiB/partition); counts (sum of
    0/1 mask, exact in bf16) are one DVE tensor_reduce (first 4096 columns)
    plus ACT activation(Copy, accum_out) chunks - both engines have slack
    under the 35 us DMA roofline.
  - tile sizes shrink geometrically (4096 -> 256) and the last mask chunk is
    fetched before the last pg tile, so the final DMA gates only the tiny
    DVE chain and the single output DMA (all partials in one fp32 tile).
Host: loss = sum(l1 / max(ct, 1)) / B.
"""

import sys

if "/opt/trn_rl_repo" not in sys.path:
    sys.path.insert(0, "/opt/trn_rl_repo")

import numpy as np

B, C, H, W = 16, 64, 128, 128
N_CORES = 8
BPC = B // N_CORES          # batches per core = 2
P = BPC * C                 # partitions per core = 128 (one (b,c) pair each)
HW = H * W                  # 16384 free elements per partition

SIZES = [4096, 4096, 2048, 2048, 1536, 1024, 768, 512, 256]   # sum = 16384
NT = len(SIZES)
OFFS = [sum(SIZES[:i]) for i in range(NT)]
N_DVE_TAIL = 2              # last tiles reduce on DVE (no ACT in the tail)

# mask chunk DMAs (lo, hi, after_pg_tile): the whole mask lands by pg1 so
# counts and the mult deps never gate the tail
MASK_CHUNKS = [
    (0, 4096, 0),
    (4096, 8192, 0),
    (8192, 12288, 1),
    (12288, 16384, 1),
]

# count chunks (lo, hi, engine, emit_after_tile); ranges must be fully
# DMA'd by their emit point
COUNT_CHUNKS = [
    (0, 4096, "act", 0),
    (4096, 8192, "dve", 1),
    (8192, 12288, "act", 2),
    (12288, 16384, "dve", 3),
]
NCC = len(COUNT_CHUNKS)

_CACHE = {}


def _build():
    key = "nc"
    if key in _CACHE:
        return _CACHE[key]

    import concourse.bacc as bacc
    import concourse.mybir as mybir
    from concourse.tile import TileContext

    f32 = mybir.dt.float32
    bf16 = mybir.dt.bfloat16
    Alu = mybir.AluOpType
    Act = mybir.ActivationFunctionType

    nc = bacc.Bacc(
        "TRN2",
        target_bir_lowering=False,
        debug=False,
        enable_asserts=False,
        num_devices=N_CORES,
    )

    pgin = nc.dram_tensor("pgin", [P, 2 * HW], f32, kind="ExternalInput").ap()
    mask = nc.dram_tensor("mask", [P, HW], f32, kind="ExternalInput").ap()
    out = nc.dram_tensor("out", [P, NT + NCC], f32, kind="ExternalOutput").ap()

    with TileContext(nc) as tc:
        with (
            tc.tile_pool(name="pg", bufs=4) as pg,
            tc.tile_pool(name="mp", bufs=1) as mp,
            tc.tile_pool(name="work", bufs=3) as work,
            tc.tile_pool(name="acc", bufs=1) as accp,
        ):
            acc = accp.tile([P, NT + NCC], f32, tag="acc")
            l1p = acc[:, 0:NT]
            ctp = acc[:, NT : NT + NCC]
            trash = accp.tile([P, 4096], bf16, tag="trash")
            tmr = mp.tile([P, HW], bf16, tag="mask")   # resident bf16 mask

            def emit_counts(after_tile):
                for ci, (lo, hi, eng, ready) in enumerate(COUNT_CHUNKS):
                    if ready != after_tile:
                        continue
                    if eng == "dve":
                        nc.vector.tensor_reduce(
                            out=ctp[:, ci : ci + 1],
                            in_=tmr[:, lo:hi],
                            axis=mybir.AxisListType.X,
                            op=Alu.add,
                        )
                    else:
                        nc.scalar.activation(
                            out=trash[:, : hi - lo],
                            in_=tmr[:, lo:hi],
                            func=Act.Copy,
                            accum_out=ctp[:, ci : ci + 1],
                        )

            for i in range(NT):
                s, o = SIZES[i], OFFS[i]

                # DMA order: pg_i, then any mask chunks scheduled after it
                # (all mask bytes land by pg1); the final DMA is the last
                # (tiny) pg tile, gating only the short DVE chain
                xt = pg.tile([P, 2 * s], bf16, tag="pg", name=f"xt{i}")
                nc.gpsimd.dma_start(out=xt, in_=pgin[:, 2 * o : 2 * o + 2 * s])
                for lo, hi, after in MASK_CHUNKS:
                    if after == i:
                        nc.gpsimd.dma_start(out=tmr[:, lo:hi], in_=mask[:, lo:hi])

                emit_counts(i)

                # DVE: two 2x-mode TTs; ACT (or DVE for tail tiles) reduces
                sd = work.tile([P, s], bf16, tag="sd")
                y = work.tile([P, s], bf16, tag="y", name=f"y{i}")
                nc.vector.tensor_tensor(
                    out=sd, in0=xt[:, 0:s], in1=xt[:, s : 2 * s], op=Alu.subtract
                )
                nc.vector.tensor_tensor(
                    out=y, in0=sd, in1=tmr[:, o : o + s], op=Alu.mult
                )
                if i < NT - N_DVE_TAIL:
                    nc.scalar.activation(
                        out=trash[:, :s],
                        in_=y,
                        func=Act.Abs,
                        accum_out=l1p[:, i : i + 1],
                    )
                else:
                    nc.vector.tensor_reduce(
                        out=l1p[:, i : i + 1],
                        in_=y,
                        axis=mybir.AxisListType.X,
                        op=Alu.add,
                        apply_absolute_value=True,
                    )

            nc.sync.dma_start(out=out, in_=acc)

    nc.compile()
    _CACHE[key] = nc
    return nc


def _shard(pre, gt, mask):
    in_maps = []
    for i in range(N_CORES):
        sl = slice(i * BPC, (i + 1) * BPC)
        p = np.ascontiguousarray(pre[sl], dtype=np.float32).reshape(P, HW)
        g = np.ascontiguousarray(gt[sl], dtype=np.float32).reshape(P, HW)
        pgin = np.empty((P, 2 * HW), dtype=np.float32)
        for s, o in zip(SIZES, OFFS):
            pgin[:, 2 * o : 2 * o + s] = p[:, o : o + s]
            pgin[:, 2 * o + s : 2 * o + 2 * s] = g[:, o : o + s]
        in_maps.append(
            {
                "pgin": pgin,
                "mask": np.ascontiguousarray(mask[sl], dtype=np.float32).reshape(P, HW),
            }
        )
    return in_maps


def _combine(results, batch_size):
    total = np.float32(0.0)
    for r in results:
        o = np.asarray(r["out"], dtype=np.float32)
        l1 = o[:, :NT].sum(axis=1, dtype=np.float32)
        ct = o[:, NT:].sum(axis=1, dtype=np.float32)
        total += (l1 / np.maximum(ct, np.float32(1.0))).sum(dtype=np.float32)
    return np.asarray(total / np.float32(int(batch_size)), dtype=np.float32)


def run(pre, gt, mask, batch_size, trace=False, **bass_kwargs):
    from concourse.bass_utils import run_bass_kernel_spmd

    nc = _build()
    in_maps = _shard(np.asarray(pre), np.asarray(gt), np.asarray(mask))
    res = run_bass_kernel_spmd(
        nc, in_maps, list(range(N_CORES)), trace=trace, **bass_kwargs
    )
    loss = _combine(res.results, batch_size)
    return loss, res


def kernel(pre, gt, mask, batch_size):
    loss, _ = run(pre, gt, mask, batch_size)
    return loss


# revision 21
# speedup vs baseline: 1.7271x; 1.0032x over previous
"""Masked L1 loss (per-(b,c) normalized) on 8 Trainium2 NeuronCores.

Layout: batch-dim data parallel. Core i takes batches [2i, 2i+2) of the
[16, 64, 128, 128] inputs -> a [128, 16384] shard (partition = (b, c) pair,
free = h*w). The kernel is DMA-bound, so everything is built around the DMA
stream:

  - pre and gt are repacked on the host into one tile-major tensor
    (per tile: [pre_i | gt_i] blocks), so each tile needs ONE pre/gt DMA
    plus one mask DMA -> 19 input DMAs. All input DMAs use the gpsimd SWDGE
    path (the only one that can downcast in flight); fewer DMAs matter
    because SWDGE descriptor generation is serial (~1 us per DMA).
  - all inputs are DMA'd with an inline f32 -> bf16 cast: HBM reads are
    unchanged but the SBUF-side stream halves and all on-chip operands are
    16-bit, which doubles DVE tensor_tensor throughput (2x mode). Loss
    tolerance is 2e-2; bf16 + fp32 accumulation gives ~1e-4.
  - per tile, DVE runs just two 2x-mode tensor_tensor ops (sd = pre - gt,
    y = sd * mask); the l1 partial sum(|y|) = sum(|d|*mask) comes from ACT
    activation(Abs, accum_out) consuming y. ACT is a pure sink - nothing
    downstream waits on it mid-stream - so the cross-engine hop never
    stalls the pipeline. The last two (small) tiles instead use the all-DVE
    tensor_reduce(add, apply_absolute_value) so the post-last-byte chain
    stays on one engine.
  - the bf16 mask stays resident in SBUF (32 KiB/partition) and is fully
    fetched by pg tile 1, in four 4096-column chunks. Counts (sum of 0/1
    mask, exact in bf16) run early over those chunks, alternating DVE
    tensor_reduce / ACT activation(Copy, accum_out) - both engines stay
    under the 35 us DMA roofline and no count gates the tail.
  - tile sizes shrink geometrically (4096 -> 256), so the final DMA gates
    only the tiny DVE chain and the single output DMA (all partials are in
    one fp32 tile).
Host: loss = sum(l1 / max(ct, 1)) / B.
"""

import sys

if "/opt/trn_rl_repo" not in sys.path:
    sys.path.insert(0, "/opt/trn_rl_repo")

import numpy as np

B, C, H, W = 16, 64, 128, 128
N_CORES = 8
BPC = B // N_CORES          # batches per core = 2
P = BPC * C                 # partitions per core = 128 (one (b,c) pair each)
HW = H * W                  # 16384 free elements per partition

SIZES = [4096, 4096, 2048, 2048, 1536, 1024, 768, 512, 256]   # sum = 16384
NT = len(SIZES)
OFFS = [sum(SIZES[:i]) for i in range(NT)]
N_DVE_TAIL = 2              # last tiles reduce on DVE (no ACT in the tail)

# mask chunk DMAs (lo, hi, after_pg_tile): the whole mask lands by pg1 so
# counts and the mult deps never gate the tail
MASK_CHUNKS = [
    (0, 4096, 0),
    (4096, 8192, 0),
    (8192, 12288, 1),
    (12288, 16384, 1),
]

# count chunks (lo, hi, engine, emit_after_tile); ranges must be fully
# DMA'd by their emit point
COUNT_CHUNKS = [
    (0, 4096, "act", 0),
    (4096, 8192, "dve", 1),
    (8192, 12288, "act", 2),
    (12288, 16384, "dve", 3),
]
NCC = len(COUNT_CHUNKS)

_CACHE = {}


def _build():
    key = "nc"
    if key in _CACHE:
        return _CACHE[key]

    import concourse.bacc as bacc
    import concourse.mybir as mybir
    from concourse.tile import TileContext

    f32 = mybir.dt.float32
    bf16 = mybir.dt.bfloat16
    Alu = mybir.AluOpType
    Act = mybir.ActivationFunctionType

    nc = bacc.Bacc(
        "TRN2",
        target_bir_lowering=False,
        debug=False,
        enable_asserts=False,
        num_devices=N_CORES,
    )

    pgin = nc.dram_tensor("pgin", [P, 2 * HW], f32, kind="ExternalInput").ap()
    mask = nc.dram_tensor("mask", [P, HW], f32, kind="ExternalInput").ap()
    out = nc.dram_tensor("out", [P, NT + NCC], f32, kind="ExternalOutput").ap()

    with TileContext(nc) as tc:
        with (
            tc.tile_pool(name="pg", bufs=4) as pg,
            tc.tile_pool(name="mp", bufs=1) as mp,
            tc.tile_pool(name="work", bufs=4) as work,
            tc.tile_pool(name="acc", bufs=1) as accp,
        ):
            acc = accp.tile([P, NT + NCC], f32, tag="acc")
            l1p = acc[:, 0:NT]
            ctp = acc[:, NT : NT + NCC]
            trash = accp.tile([P, 4096], bf16, tag="trash")
            tmr = mp.tile([P, HW], bf16, tag="mask")   # resident bf16 mask

            def emit_counts(after_tile):
                for ci, (lo, hi, eng, ready) in enumerate(COUNT_CHUNKS):
                    if ready != after_tile:
                        continue
                    if eng == "dve":
                        nc.vector.tensor_reduce(
                            out=ctp[:, ci : ci + 1],
                            in_=tmr[:, lo:hi],
                            axis=mybir.AxisListType.X,
                            op=Alu.add,
                        )
                    else:
                        nc.scalar.activation(
                            out=trash[:, : hi - lo],
                            in_=tmr[:, lo:hi],
                            func=Act.Copy,
                            accum_out=ctp[:, ci : ci + 1],
                        )

            for i in range(NT):
                s, o = SIZES[i], OFFS[i]

                # DMA order: pg_i, then any mask chunks scheduled after it
                # (all mask bytes land by pg1); the final DMA is the last
                # (tiny) pg tile, gating only the short DVE chain
                xt = pg.tile([P, 2 * s], bf16, tag="pg", name=f"xt{i}")
                nc.gpsimd.dma_start(out=xt, in_=pgin[:, 2 * o : 2 * o + 2 * s])
                for lo, hi, after in MASK_CHUNKS:
                    if after == i:
                        nc.gpsimd.dma_start(out=tmr[:, lo:hi], in_=mask[:, lo:hi])

                emit_counts(i)

                # DVE: two 2x-mode TTs; ACT (or DVE for tail tiles) reduces
                sd = work.tile([P, s], bf16, tag="sd")
                y = work.tile([P, s], bf16, tag="y", name=f"y{i}")
                nc.vector.tensor_tensor(
                    out=sd, in0=xt[:, 0:s], in1=xt[:, s : 2 * s], op=Alu.subtract
                )
                nc.vector.tensor_tensor(
                    out=y, in0=sd, in1=tmr[:, o : o + s], op=Alu.mult
                )
                if i < NT - N_DVE_TAIL:
                    nc.scalar.activation(
                        out=trash[:, :s],
                        in_=y,
                        func=Act.Abs,
                        accum_out=l1p[:, i : i + 1],
                    )
                else:
                    nc.vector.tensor_reduce(
                        out=l1p[:, i : i + 1],
                        in_=y,
                        axis=mybir.AxisListType.X,
                        op=Alu.add,
                        apply_absolute_value=True,
                    )

            nc.sync.dma_start(out=out, in_=acc)

    nc.compile()
    _CACHE[key] = nc
    return nc


def _shard(pre, gt, mask):
    in_maps = []
    for i in range(N_CORES):
        sl = slice(i * BPC, (i + 1) * BPC)
        p = np.ascontiguousarray(pre[sl], dtype=np.float32).reshape(P, HW)
        g = np.ascontiguousarray(gt[sl], dtype=np.float32).reshape(P, HW)
        pgin = np.empty((P, 2 * HW), dtype=np.float32)
        for s, o in zip(SIZES, OFFS):
            pgin[:, 2 * o : 2 * o + s] = p[:, o : o + s]
            pgin[:, 2 * o + s : 2 * o + 2 * s] = g[:, o : o + s]
        in_maps.append(
            {
                "pgin": pgin,
                "mask": np.ascontiguousarray(mask[sl], dtype=np.float32).reshape(P, HW),
            }
        )
    return in_maps


def _combine(results, batch_size):
    total = np.float32(0.0)
    for r in results:
        o = np.asarray(r["out"], dtype=np.float32)
        l1 = o[:, :NT].sum(axis=1, dtype=np.float32)
        ct = o[:, NT:].sum(axis=1, dtype=np.float32)
        total += (l1 / np.maximum(ct, np.float32(1.0))).sum(dtype=np.float32)
    return np.asarray(total / np.float32(int(batch_size)), dtype=np.float32)


def run(pre, gt, mask, batch_size, trace=False, **bass_kwargs):
    from concourse.bass_utils import run_bass_kernel_spmd

    nc = _build()
    in_maps = _shard(np.asarray(pre), np.asarray(gt), np.asarray(mask))
    res = run_bass_kernel_spmd(
        nc, in_maps, list(range(N_CORES)), trace=trace, **bass_kwargs
    )
    loss = _combine(res.results, batch_size)
    return loss, res


def kernel(pre, gt, mask, batch_size):
    loss, _ = run(pre, gt, mask, batch_size)
    return loss
